# revision 30
# baseline (speedup 1.0000x reference)
"""DeepSeekMoE layer on 8 Trainium2 NeuronCores (expert-parallel).

Hardcoded problem shapes: B=2, S=1024, D=2048, I=1408, E=8, TOPK=2.

Sharding: core e holds routed expert e (dense over all 2048 tokens)
plus a 176-wide I-slice of the shared expert (zero-padded to 256).
Routed output is pre-scaled by the per-token combine weight (folded
into the SwiGLU product), so routed and shared partials accumulate in
the same PSUM; ReduceScatters over the 8 cores (one per 512-token
quarter, overlapped with compute) sum the partials and hand core j its
token rows. The router runs replicated on every core in true fp32 so
the top-2 selection matches the fp32 reference bit-for-bit; the FFN
runs in bf16 with fp32 PSUM accumulation.

Host side only does value-independent layout prep (weight transposes /
tiling / bf16 casts, x transpose) and shard concatenation.
"""

import numpy as np
import ml_dtypes

import concourse.bacc as bacc
import concourse.mybir as mybir
import concourse.tile as tile
from concourse.bass_utils import run_bass_kernel_spmd
from concourse.masks import make_identity

F32 = mybir.dt.float32
BF16 = mybir.dt.bfloat16
I32 = mybir.dt.int32
U32 = mybir.dt.uint32
BF = ml_dtypes.bfloat16

B, S, D, I, E, TOPK = 2, 1024, 2048, 1408, 8, 2
N = B * S                    # 2048 tokens
NCORES = 8
IS = I // NCORES             # shared I slice per core (176)
ISP = 256                    # padded shared slice
NH = 2                       # token halves
NT = N // NH                 # tokens per half (1024)
TT = NT // 128               # token tiles per half (8)
DT = D // 128                # 16
IT = I // 128                # 11 (routed i tiles)
IST = ISP // 128             # 2  (shared i tiles, padded)
NQ = 4                       # reduce-scatter quarters (512 tokens each)
QR = N // NQ // NCORES       # rows per core per quarter (64)

_CACHE = {}


def _build():
    if "nc" in _CACHE:
        return _CACHE["nc"]

    nc = bacc.Bacc("TRN2", target_bir_lowering=False, debug=False,
                   num_devices=NCORES)

    xLbf = nc.dram_tensor("xLbf", [NH * 128, DT * NT], BF16, kind="ExternalInput")
    xro = nc.dram_tensor("xro", [128, DT * 256], F32, kind="ExternalInput")
    ecol = nc.dram_tensor("ecol", [1, 1], F32, kind="ExternalInput")
    wgT = nc.dram_tensor("wgT", [D, E], F32, kind="ExternalInput")
    ebias = nc.dram_tensor("ebias", [1, E], F32, kind="ExternalInput")
    esel = nc.dram_tensor("esel", [1, E], F32, kind="ExternalInput")
    iota8 = nc.dram_tensor("iota8", [1, E], F32, kind="ExternalInput")
    # routed expert weights, i-tile-major bf16 layouts
    weL = nc.dram_tensor("weL", [IT * 128, DT * 128], BF16, kind="ExternalInput")
    wuL = nc.dram_tensor("wuL", [IT * 128, DT * 128], BF16, kind="ExternalInput")
    wdL = nc.dram_tensor("wdL", [IT * 128, D], BF16, kind="ExternalInput")
    # shared expert slice (padded to ISP rows)
    wsgL = nc.dram_tensor("wsgL", [IST * 128, DT * 128], BF16, kind="ExternalInput")
    wsuL = nc.dram_tensor("wsuL", [IST * 128, DT * 128], BF16, kind="ExternalInput")
    wsdL = nc.dram_tensor("wsdL", [IST * 128, D], BF16, kind="ExternalInput")

    out_shard = nc.dram_tensor("out_shard", [N // NCORES, D], F32,
                               kind="ExternalOutput")
    out_idx = nc.dram_tensor("out_idx", [N, TOPK], I32, kind="ExternalOutput")

    ACT = mybir.ActivationFunctionType
    ALU = mybir.AluOpType

    with tile.TileContext(nc) as tc:
        with (
            tc.tile_pool(name="big", bufs=1) as big,
            tc.tile_pool(name="xbp", bufs=2) as xbp,
            tc.tile_pool(name="wstr", bufs=3) as wstr,
            tc.tile_pool(name="wdp", bufs=8) as wdp,
            tc.tile_pool(name="ysp", bufs=4) as ysp,
            tc.tile_pool(name="sm", bufs=2) as sm,
            tc.tile_pool(name="psf", bufs=2, space="PSUM") as psf,
            tc.tile_pool(name="psy", bufs=1, space="PSUM") as psy,
            tc.tile_pool(name="dram", bufs=1, space="DRAM") as dram,
        ):
            ident = big.tile([128, 128], F32, tag="ident")
            make_identity(nc, ident[:])
            bias_b = big.tile([128, E], F32, tag="bias_b")
            nc.sync.dma_start(out=bias_b[:], in_=ebias[:, :].to_broadcast((128, E)))
            esel_b = big.tile([128, E], F32, tag="esel_b")
            nc.sync.dma_start(out=esel_b[:], in_=esel[:, :].to_broadcast((128, E)))
            iota_b = big.tile([128, E], F32, tag="iota_b")
            nc.sync.dma_start(out=iota_b[:], in_=iota8[:, :].to_broadcast((128, E)))
            wg_sb = big.tile([128, DT, E], F32, tag="wg_sb")
            nc.sync.dma_start(
                out=wg_sb[:],
                in_=wgT[:, :].rearrange("(k p) e -> p k e", p=128))

            G_ROWS = [512, 512, 512, 256, 256]
            rs_in = [dram.tile([G_ROWS[q], D], F32, tag=f"rs_in{q}",
                               name=f"rs_in{q}") for q in range(5)]
            ag_in = dram.tile([256, 4], F32, tag="ag_in", name="ag_in")
            ag_out = dram.tile([N, 4], F32, tag="ag_out", name="ag_out")

            # ---- sharded router: fp32 scores for own 256 tokens ----
            ec_b = big.tile([128, 1], F32, tag="ec_b")
            nc.sync.dma_start(out=ec_b[:], in_=ecol[:, :].to_broadcast((128, 1)))
            xro_sb = big.tile([128, DT, 256], F32, tag="xro_sb")
            nc.sync.dma_start(
                out=xro_sb[:],
                in_=xro[:, :].rearrange("p (k j) -> p k j", k=DT))
            zps = psf.tile([E, 256], F32, tag="pg")
            for k in range(DT):
                nc.tensor.matmul(out=zps[:], lhsT=wg_sb[:, k, :],
                                 rhs=xro_sb[:, k, :],
                                 start=(k == 0), stop=(k == DT - 1))
            zsb = sm.tile([E, 256], F32, tag="zsb")
            nc.scalar.activation(out=zsb[:], in_=zps[:], func=ACT.Sigmoid)
            pack = sm.tile([128, 2, 4], F32, tag="pack")
            for t in range(2):
                tp = psf.tile([128, E], F32, tag="pu")
                nc.tensor.transpose(out=tp[:],
                                    in_=zsb[:, t * 128:(t + 1) * 128],
                                    identity=ident[:E, :E])
                sc = sm.tile([128, E], F32, tag="sc")
                nc.vector.tensor_copy(out=sc[:], in_=tp[:])
                sel = sm.tile([128, E], F32, tag="sel")
                nc.vector.tensor_add(out=sel[:], in0=sc[:], in1=bias_b[:])
                mx = sm.tile([128, E], F32, tag="mx")
                mi = sm.tile([128, E], U32, tag="mi")
                nc.vector.max(out=mx[:], in_=sel[:])
                nc.vector.max_index(out=mi[:], in_max=mx[:], in_values=sel[:])
                mif = sm.tile([128, E], F32, tag="mif")
                nc.vector.tensor_copy(out=mif[:], in_=mi[:])
                nc.vector.tensor_copy(out=pack[:, t, 0:2], in_=mif[:, 0:2])
                oh1 = sm.tile([128, E], F32, tag="oh1")
                oh2 = sm.tile([128, E], F32, tag="oh2")
                nc.vector.tensor_tensor(
                    out=oh1[:], in0=mif[:, 0:1].to_broadcast((128, E)),
                    in1=iota_b[:], op=ALU.is_equal)
                nc.vector.tensor_tensor(
                    out=oh2[:], in0=mif[:, 1:2].to_broadcast((128, E)),
                    in1=iota_b[:], op=ALU.is_equal)
                prod = sm.tile([128, E], F32, tag="prod")
                w1 = sm.tile([128, 1], F32, tag="w1")
                w2 = sm.tile([128, 1], F32, tag="w2")
                nc.vector.tensor_mul(out=prod[:], in0=oh1[:], in1=sc[:])
                nc.vector.tensor_reduce(out=w1[:], in_=prod[:],
                                        axis=mybir.AxisListType.X, op=ALU.add)
                nc.vector.tensor_mul(out=prod[:], in0=oh2[:], in1=sc[:])
                nc.vector.tensor_reduce(out=w2[:], in_=prod[:],
                                        axis=mybir.AxisListType.X, op=ALU.add)
                ssum = sm.tile([128, 1], F32, tag="ssum")
                nc.vector.tensor_add(out=ssum[:], in0=w1[:], in1=w2[:])
                sinv = sm.tile([128, 1], F32, tag="sinv")
                nc.vector.reciprocal(out=sinv[:], in_=ssum[:])
                nc.vector.tensor_mul(out=pack[:, t, 2:3], in0=w1[:], in1=sinv[:])
                nc.vector.tensor_mul(out=pack[:, t, 3:4], in0=w2[:], in1=sinv[:])
            nc.scalar.dma_start(
                out=ag_in[:].rearrange("(t p) c -> p t c", p=128),
                in_=pack[:])
            nc.gpsimd.collective_compute(
                "AllGather", ALU.bypass,
                replica_groups=[list(range(NCORES))],
                ins=[ag_in[:].opt()],
                outs=[ag_out[:].opt()])
            pk = sm.tile([128, N // 128, 4], F32, tag="pk")
            nc.sync.dma_start(
                out=pk[:],
                in_=ag_out[:].rearrange("(t p) c -> p t c", p=128))
            comb_all = big.tile([128, N // 128], F32, tag="comb_all")
            idx_all = sm.tile([128, N // 128, TOPK], I32, tag="idx_all")
            for t in range(N // 128):
                eq1 = sm.tile([128, 1], F32, tag="eq1")
                eq2 = sm.tile([128, 1], F32, tag="eq2")
                nc.vector.tensor_scalar(out=eq1[:], in0=pk[:, t, 0:1],
                                        scalar1=ec_b[:], scalar2=None,
                                        op0=ALU.is_equal)
                nc.vector.tensor_scalar(out=eq2[:], in0=pk[:, t, 1:2],
                                        scalar1=ec_b[:], scalar2=None,
                                        op0=ALU.is_equal)
                m1 = sm.tile([128, 1], F32, tag="m1")
                m2 = sm.tile([128, 1], F32, tag="m2")
                nc.vector.tensor_mul(out=m1[:], in0=eq1[:], in1=pk[:, t, 2:3])
                nc.vector.tensor_mul(out=m2[:], in0=eq2[:], in1=pk[:, t, 3:4])
                nc.vector.tensor_add(out=comb_all[:, t:t + 1], in0=m1[:],
                                     in1=m2[:])
                nc.vector.tensor_copy(out=idx_all[:, t, :], in_=pk[:, t, 0:2])
            nc.sync.dma_start(
                out=out_idx[:, :].rearrange("(t p) k -> p t k", p=128),
                in_=idx_all[:])
            rs_out = [dram.tile([G_ROWS[q] // NCORES, D], F32,
                                tag=f"rs_out{q}",
                                name=f"rs_out{q}") for q in range(5)]

            for h in range(NH):
                # ---- load x half (bf16 only; router is pre-computed) ----
                xbf_t = [xbp.tile([128, 4, NT], BF16, tag=f"xbf{k4}",
                                  name=f"xbf{h}_{k4}") for k4 in range(4)]
                for k4 in range(4):
                    nc.sync.dma_start(
                        out=xbf_t[k4][:],
                        in_=xLbf[h * 128:(h + 1) * 128,
                                 k4 * 4 * NT:(k4 + 1) * 4 * NT].rearrange(
                            "p (k j) -> p k j", k=4))

                def xbf(k, cs):
                    return xbf_t[k // 4][:, k % 4, cs]

                # ---- gate/up + swiglu -> hh (bf16; routed comb-scaled) ----
                hh = big.tile([128, IT, NT], BF16, tag="hh")
                hhs = big.tile([128, IST, NT], BF16, tag="hhs")
                for i in range(IT + IST):
                    shared = i >= IT
                    isl = i - IT if shared else i
                    gsrc, usrc = (wsgL, wsuL) if shared else (weL, wuL)
                    wg_t = wstr.tile([128, DT, 128], BF16, tag="wg_t")
                    wu_t = wstr.tile([128, DT, 128], BF16, tag="wu_t")
                    rsl = slice(isl * 128, (isl + 1) * 128)
                    for k4 in range(4):
                        ks = slice(k4 * 4, (k4 + 1) * 4)
                        csl = slice(k4 * 4 * 128, (k4 + 1) * 4 * 128)
                        nc.sync.dma_start(
                            out=wg_t[:, ks, :],
                            in_=gsrc[rsl, csl].rearrange("p (k j) -> p k j", k=4))
                        nc.sync.dma_start(
                            out=wu_t[:, ks, :],
                            in_=usrc[rsl, csl].rearrange("p (k j) -> p k j", k=4))
                    for c in range(NT // 512):
                        cs = slice(c * 512, (c + 1) * 512)
                        pg = psf.tile([128, 512], F32, tag="pg")
                        pu = psf.tile([128, 512], F32, tag="pu")
                        for k in range(DT):
                            nc.tensor.matmul(out=pg[:],
                                             lhsT=wg_t[:, k, :],
                                             rhs=xbf(k, cs),
                                             start=(k == 0), stop=(k == DT - 1))
                        for k in range(DT):
                            nc.tensor.matmul(out=pu[:],
                                             lhsT=wu_t[:, k, :],
                                             rhs=xbf(k, cs),
                                             start=(k == 0), stop=(k == DT - 1))
                        hg = sm.tile([128, 512], F32, tag="hg")
                        nc.scalar.activation(out=hg[:], in_=pg[:],
                                             func=ACT.Silu)
                        dst = hhs if shared else hh
                        nc.vector.tensor_mul(out=dst[:, isl, cs],
                                             in0=hg[:], in1=pu[:])

                # ---- down-proj per 512-token group; RS per quarter ----
                half_groups = [(0, 4), (4, 4)] if h == 0 else \
                    [(0, 4), (4, 2), (6, 2)]
                for grp, (t0, ntt) in enumerate(half_groups):
                    q = (h * 2 + grp) if h == 0 else (2 + grp)
                    tts = tuple(range(ntt))
                    if True:
                      for c in range(D // 512):
                        cs = slice(c * 512, (c + 1) * 512)
                        # shared expert slice -> ysh
                        sps = [psy.tile([128, 512], F32, tag=f"y{t}",
                                        name=f"s_{q}_{c}_{t}")
                               for t in tts]
                        for isl in range(IST):
                            wd_s = wdp.tile([128, 512], BF16, tag="wd_t")
                            nc.gpsimd.dma_start(
                                out=wd_s[:],
                                in_=wsdL[isl * 128:(isl + 1) * 128, cs])
                            for j, tt in enumerate(tts):
                                t = t0 + tt
                                nc.tensor.matmul(
                                    out=sps[j][:],
                                    lhsT=hhs[:, isl, t * 128:(t + 1) * 128],
                                    rhs=wd_s[:],
                                    start=(isl == 0), stop=(isl == IST - 1))
                        yshs = []
                        for j, tt in enumerate(tts):
                            ysh = ysp.tile([128, 512], F32, tag=f"ysh{tt}",
                                           name=f"ysh_{q}_{c}_{tt}")
                            nc.vector.tensor_copy(out=ysh[:], in_=sps[j][:])
                            yshs.append(ysh)
                        # routed expert, comb-scaled at drain, += shared
                        yps = [psy.tile([128, 512], F32, tag=f"y{t}",
                                        name=f"y_{q}_{c}_{t}")
                               for t in tts]
                        for isl in range(IT):
                            wd_t = wdp.tile([128, 512], BF16, tag="wd_t")
                            nc.gpsimd.dma_start(
                                out=wd_t[:],
                                in_=wdL[isl * 128:(isl + 1) * 128, cs])
                            for j, tt in enumerate(tts):
                                t = t0 + tt
                                nc.tensor.matmul(
                                    out=yps[j][:],
                                    lhsT=hh[:, isl, t * 128:(t + 1) * 128],
                                    rhs=wd_t[:],
                                    start=(isl == 0), stop=(isl == IT - 1))
                        for j, tt in enumerate(tts):
                            t = t0 + tt
                            ysb = ysp.tile([128, 512], F32, tag="ysb")
                            ta = h * TT + t
                            nc.vector.scalar_tensor_tensor(
                                out=ysb[:], in0=yps[j][:],
                                scalar=comb_all[:, ta:ta + 1], in1=yshs[j][:],
                                op0=mybir.AluOpType.mult,
                                op1=mybir.AluOpType.add)
                            nc.scalar.dma_start(
                                out=rs_in[q][tt * 128:(tt + 1) * 128, cs],
                                in_=ysb[:])
                    if True:
                        nc.gpsimd.collective_compute(
                            "ReduceScatter",
                            ALU.add,
                            replica_groups=[list(range(NCORES))],
                            ins=[rs_in[q][:].opt()],
                            outs=[rs_out[q][:].opt()],
                        )

            row = 0
            for q in range(5):
                gr = G_ROWS[q] // NCORES
                nc.sync.dma_start(
                    out=out_shard[row:row + gr, :],
                    in_=rs_out[q][:])
                row += gr

    nc.compile()
    _CACHE["nc"] = nc
    return nc


def _tile_gateup(wT_bf):
    # [D, Icols] bf16 -> [Icols(it-major), DT*128] st. row it*128+j, col k*128+p
    # equals wT[k*128+p, it*128+j]
    d, icols = wT_bf.shape
    it = icols // 128
    a = wT_bf.reshape(DT, 128, it, 128).transpose(2, 1, 0, 3)  # [it, p, k, j]
    return np.ascontiguousarray(a.reshape(it * 128, DT * 128))


def _make_in_maps(x, gate_w, expert_bias, shared_gate, shared_up, shared_down,
                  exp_gate, exp_up, exp_down):
    x2 = np.ascontiguousarray(np.asarray(x, np.float32).reshape(N, D))
    xT_v = x2.T  # [D, N]
    # xL[h*128+p, k*NT+j] = xT[k*128+p, h*NT+j]
    xL_v = np.ascontiguousarray(
        xT_v.reshape(DT, 128, NH, NT).transpose(2, 1, 0, 3).reshape(
            NH * 128, DT * NT))
    xLbf_v = xL_v.astype(BF)
    wgT_v = np.ascontiguousarray(np.asarray(gate_w, np.float32).T)
    ebias_v = np.ascontiguousarray(np.asarray(expert_bias, np.float32)
                                   .reshape(1, E))
    iota_v = np.arange(E, dtype=np.float32).reshape(1, E)

    sg = np.asarray(shared_gate, np.float32)
    su = np.asarray(shared_up, np.float32)
    sd = np.asarray(shared_down, np.float32)

    in_maps = []
    for e in range(NCORES):
        sl = slice(e * IS, (e + 1) * IS)
        esel_v = np.zeros((1, E), np.float32)
        esel_v[0, e] = 1.0

        weT = np.asarray(exp_gate[e], np.float32).T.astype(BF)   # [D, I]
        wuT = np.asarray(exp_up[e], np.float32).T.astype(BF)
        wdT = np.asarray(exp_down[e], np.float32).T.astype(BF)   # [I, D]

        wsgT = np.zeros((D, ISP), BF)
        wsgT[:, :IS] = sg[sl, :].T.astype(BF)
        wsuT = np.zeros((D, ISP), BF)
        wsuT[:, :IS] = su[sl, :].T.astype(BF)
        wsd_p = np.zeros((ISP, D), BF)
        wsd_p[:IS, :] = sd[:, sl].T.astype(BF)

        xro_v = np.ascontiguousarray(
            xT_v[:, e * 256:(e + 1) * 256].reshape(DT, 128, 256)
            .transpose(1, 0, 2).reshape(128, DT * 256))
        in_maps.append({
            "xro": xro_v,
            "ecol": np.full((1, 1), float(e), np.float32),
            "xLbf": xLbf_v,
            "wgT": wgT_v,
            "ebias": ebias_v,
            "esel": esel_v,
            "iota8": iota_v,
            "weL": _tile_gateup(weT),
            "wuL": _tile_gateup(wuT),
            "wdL": np.ascontiguousarray(wdT),
            "wsgL": _tile_gateup(wsgT),
            "wsuL": _tile_gateup(wsuT),
            "wsdL": np.ascontiguousarray(wsd_p),
        })
    return in_maps


def _assemble(results):
    out = np.empty((N, D), np.float32)
    G_ROWS = [512, 512, 512, 256, 256]
    base_t = 0
    row = 0
    for q in range(5):
        gr = G_ROWS[q] // NCORES
        for r in range(NCORES):
            out[base_t + r * gr:base_t + (r + 1) * gr, :] = \
                results[r]["out_shard"][row:row + gr, :]
        base_t += G_ROWS[q]
        row += gr
    idx = results[0]["out_idx"].astype(np.int32)
    return out.reshape(B, S, D), idx.reshape(B, S, TOPK)


def kernel(x, gate_w, expert_bias, shared_gate, shared_up, shared_down,
           exp_gate, exp_up, exp_down, _trace=False):
    nc = _build()
    in_maps = _make_in_maps(x, gate_w, expert_bias, shared_gate, shared_up,
                            shared_down, exp_gate, exp_up, exp_down)
    last_err = None
    for attempt in range(3):
        try:
            res = run_bass_kernel_spmd(nc, in_maps,
                                       core_ids=list(range(NCORES)),
                                       trace=_trace)
            break
        except Exception as e:  # transient NRT_EXEC_UNIT_UNRECOVERABLE
            last_err = e
            if "UNRECOVERABLE" not in str(e) and "UNAVAILABLE" not in str(e):
                raise
            import time
            time.sleep(10)
    else:
        raise last_err
    _CACHE["last_result"] = res
    return _assemble(res.results)


# revision 31
# speedup vs baseline: 1.0337x; 1.0337x over previous
"""DeepSeekMoE layer on 8 Trainium2 NeuronCores (expert-parallel).

Hardcoded problem shapes: B=2, S=1024, D=2048, I=1408, E=8, TOPK=2.

Sharding: core e holds routed expert e (dense over all 2048 tokens)
plus a 176-wide I-slice of the shared expert (zero-padded to 256).
Routed output is pre-scaled by the per-token combine weight (folded
into the SwiGLU product), so routed and shared partials accumulate in
the same PSUM; ReduceScatters over the 8 cores (one per 512-token
quarter, overlapped with compute) sum the partials and hand core j its
token rows. The router runs replicated on every core in true fp32 so
the top-2 selection matches the fp32 reference bit-for-bit; the FFN
runs in bf16 with fp32 PSUM accumulation.

Host side only does value-independent layout prep (weight transposes /
tiling / bf16 casts, x transpose) and shard concatenation.
"""

import numpy as np
import ml_dtypes

import concourse.bacc as bacc
import concourse.mybir as mybir
import concourse.tile as tile
from concourse.bass_utils import run_bass_kernel_spmd
from concourse.masks import make_identity

F32 = mybir.dt.float32
BF16 = mybir.dt.bfloat16
I32 = mybir.dt.int32
U32 = mybir.dt.uint32
BF = ml_dtypes.bfloat16

B, S, D, I, E, TOPK = 2, 1024, 2048, 1408, 8, 2
N = B * S                    # 2048 tokens
NCORES = 8
IS = I // NCORES             # shared I slice per core (176)
ISP = 256                    # padded shared slice
NH = 2                       # token halves
NT = N // NH                 # tokens per half (1024)
TT = NT // 128               # token tiles per half (8)
DT = D // 128                # 16
IT = I // 128                # 11 (routed i tiles)
IST = ISP // 128             # 2  (shared i tiles, padded)
NQ = 4                       # reduce-scatter quarters (512 tokens each)
QR = N // NQ // NCORES       # rows per core per quarter (64)

_CACHE = {}


def _build():
    if "nc" in _CACHE:
        return _CACHE["nc"]

    nc = bacc.Bacc("TRN2", target_bir_lowering=False, debug=False,
                   num_devices=NCORES)

    xLbf = nc.dram_tensor("xLbf", [NH * 128, DT * NT], BF16, kind="ExternalInput")
    xro = nc.dram_tensor("xro", [128, DT * 256], F32, kind="ExternalInput")
    ecol = nc.dram_tensor("ecol", [1, 1], F32, kind="ExternalInput")
    wgT = nc.dram_tensor("wgT", [D, E], F32, kind="ExternalInput")
    ebias = nc.dram_tensor("ebias", [1, E], F32, kind="ExternalInput")
    esel = nc.dram_tensor("esel", [1, E], F32, kind="ExternalInput")
    iota8 = nc.dram_tensor("iota8", [1, E], F32, kind="ExternalInput")
    # routed expert weights, i-tile-major bf16 layouts
    weL = nc.dram_tensor("weL", [IT * 128, DT * 128], BF16, kind="ExternalInput")
    wuL = nc.dram_tensor("wuL", [IT * 128, DT * 128], BF16, kind="ExternalInput")
    wdL = nc.dram_tensor("wdL", [IT * 128, D], BF16, kind="ExternalInput")
    # shared expert slice (padded to ISP rows)
    wsgL = nc.dram_tensor("wsgL", [IST * 128, DT * 128], BF16, kind="ExternalInput")
    wsuL = nc.dram_tensor("wsuL", [IST * 128, DT * 128], BF16, kind="ExternalInput")
    wsdL = nc.dram_tensor("wsdL", [IST * 128, D], BF16, kind="ExternalInput")

    out_shard = nc.dram_tensor("out_shard", [N // NCORES, D], F32,
                               kind="ExternalOutput")
    out_idx = nc.dram_tensor("out_idx", [N, TOPK], I32, kind="ExternalOutput")

    ACT = mybir.ActivationFunctionType
    ALU = mybir.AluOpType

    with tile.TileContext(nc) as tc:
        with (
            tc.tile_pool(name="big", bufs=1) as big,
            tc.tile_pool(name="xbp", bufs=2) as xbp,
            tc.tile_pool(name="wstr", bufs=3) as wstr,
            tc.tile_pool(name="wdp", bufs=8) as wdp,
            tc.tile_pool(name="ysp", bufs=4) as ysp,
            tc.tile_pool(name="sm", bufs=2) as sm,
            tc.tile_pool(name="psf", bufs=2, space="PSUM") as psf,
            tc.tile_pool(name="psy", bufs=1, space="PSUM") as psy,
            tc.tile_pool(name="dram", bufs=1, space="DRAM") as dram,
        ):
            ident = big.tile([128, 128], F32, tag="ident")
            make_identity(nc, ident[:])
            bias_b = big.tile([128, E], F32, tag="bias_b")
            nc.sync.dma_start(out=bias_b[:], in_=ebias[:, :].to_broadcast((128, E)))
            esel_b = big.tile([128, E], F32, tag="esel_b")
            nc.sync.dma_start(out=esel_b[:], in_=esel[:, :].to_broadcast((128, E)))
            iota_b = big.tile([128, E], F32, tag="iota_b")
            nc.sync.dma_start(out=iota_b[:], in_=iota8[:, :].to_broadcast((128, E)))
            wg_sb = big.tile([128, DT, E], F32, tag="wg_sb")
            nc.sync.dma_start(
                out=wg_sb[:],
                in_=wgT[:, :].rearrange("(k p) e -> p k e", p=128))

            rs_in = [dram.tile([N // NQ, D], F32, tag=f"rs_in{q}",
                               name=f"rs_in{q}") for q in range(NQ)]
            ag_in = dram.tile([256, 4], F32, tag="ag_in", name="ag_in")
            ag_out = dram.tile([N, 4], F32, tag="ag_out", name="ag_out")

            # ---- sharded router: fp32 scores for own 256 tokens ----
            ec_b = big.tile([128, 1], F32, tag="ec_b")
            nc.sync.dma_start(out=ec_b[:], in_=ecol[:, :].to_broadcast((128, 1)))
            xro_sb = big.tile([128, DT, 256], F32, tag="xro_sb")
            nc.sync.dma_start(
                out=xro_sb[:],
                in_=xro[:, :].rearrange("p (k j) -> p k j", k=DT))
            zps = psf.tile([E, 256], F32, tag="pg")
            for k in range(DT):
                nc.tensor.matmul(out=zps[:], lhsT=wg_sb[:, k, :],
                                 rhs=xro_sb[:, k, :],
                                 start=(k == 0), stop=(k == DT - 1))
            zsb = sm.tile([E, 256], F32, tag="zsb")
            nc.scalar.activation(out=zsb[:], in_=zps[:], func=ACT.Sigmoid)
            pack = sm.tile([128, 2, 4], F32, tag="pack")
            for t in range(2):
                tp = psf.tile([128, E], F32, tag="pu")
                nc.tensor.transpose(out=tp[:],
                                    in_=zsb[:, t * 128:(t + 1) * 128],
                                    identity=ident[:E, :E])
                sc = sm.tile([128, E], F32, tag="sc")
                nc.vector.tensor_copy(out=sc[:], in_=tp[:])
                sel = sm.tile([128, E], F32, tag="sel")
                nc.vector.tensor_add(out=sel[:], in0=sc[:], in1=bias_b[:])
                mx = sm.tile([128, E], F32, tag="mx")
                mi = sm.tile([128, E], U32, tag="mi")
                nc.vector.max(out=mx[:], in_=sel[:])
                nc.vector.max_index(out=mi[:], in_max=mx[:], in_values=sel[:])
                mif = sm.tile([128, E], F32, tag="mif")
                nc.vector.tensor_copy(out=mif[:], in_=mi[:])
                nc.vector.tensor_copy(out=pack[:, t, 0:2], in_=mif[:, 0:2])
                oh1 = sm.tile([128, E], F32, tag="oh1")
                oh2 = sm.tile([128, E], F32, tag="oh2")
                nc.vector.tensor_tensor(
                    out=oh1[:], in0=mif[:, 0:1].to_broadcast((128, E)),
                    in1=iota_b[:], op=ALU.is_equal)
                nc.vector.tensor_tensor(
                    out=oh2[:], in0=mif[:, 1:2].to_broadcast((128, E)),
                    in1=iota_b[:], op=ALU.is_equal)
                prod = sm.tile([128, E], F32, tag="prod")
                w1 = sm.tile([128, 1], F32, tag="w1")
                w2 = sm.tile([128, 1], F32, tag="w2")
                nc.vector.tensor_mul(out=prod[:], in0=oh1[:], in1=sc[:])
                nc.vector.tensor_reduce(out=w1[:], in_=prod[:],
                                        axis=mybir.AxisListType.X, op=ALU.add)
                nc.vector.tensor_mul(out=prod[:], in0=oh2[:], in1=sc[:])
                nc.vector.tensor_reduce(out=w2[:], in_=prod[:],
                                        axis=mybir.AxisListType.X, op=ALU.add)
                ssum = sm.tile([128, 1], F32, tag="ssum")
                nc.vector.tensor_add(out=ssum[:], in0=w1[:], in1=w2[:])
                sinv = sm.tile([128, 1], F32, tag="sinv")
                nc.vector.reciprocal(out=sinv[:], in_=ssum[:])
                nc.vector.tensor_mul(out=pack[:, t, 2:3], in0=w1[:], in1=sinv[:])
                nc.vector.tensor_mul(out=pack[:, t, 3:4], in0=w2[:], in1=sinv[:])
            nc.scalar.dma_start(
                out=ag_in[:].rearrange("(t p) c -> p t c", p=128),
                in_=pack[:])
            nc.gpsimd.collective_compute(
                "AllGather", ALU.bypass,
                replica_groups=[list(range(NCORES))],
                ins=[ag_in[:].opt()],
                outs=[ag_out[:].opt()])
            pk = sm.tile([128, N // 128, 4], F32, tag="pk")
            nc.sync.dma_start(
                out=pk[:],
                in_=ag_out[:].rearrange("(t p) c -> p t c", p=128))
            comb_all = big.tile([128, N // 128], F32, tag="comb_all")
            idx_all = sm.tile([128, N // 128, TOPK], I32, tag="idx_all")
            for t in range(N // 128):
                eq1 = sm.tile([128, 1], F32, tag="eq1")
                eq2 = sm.tile([128, 1], F32, tag="eq2")
                nc.vector.tensor_scalar(out=eq1[:], in0=pk[:, t, 0:1],
                                        scalar1=ec_b[:], scalar2=None,
                                        op0=ALU.is_equal)
                nc.vector.tensor_scalar(out=eq2[:], in0=pk[:, t, 1:2],
                                        scalar1=ec_b[:], scalar2=None,
                                        op0=ALU.is_equal)
                m1 = sm.tile([128, 1], F32, tag="m1")
                m2 = sm.tile([128, 1], F32, tag="m2")
                nc.vector.tensor_mul(out=m1[:], in0=eq1[:], in1=pk[:, t, 2:3])
                nc.vector.tensor_mul(out=m2[:], in0=eq2[:], in1=pk[:, t, 3:4])
                nc.vector.tensor_add(out=comb_all[:, t:t + 1], in0=m1[:],
                                     in1=m2[:])
                nc.vector.tensor_copy(out=idx_all[:, t, :], in_=pk[:, t, 0:2])
            nc.sync.dma_start(
                out=out_idx[:, :].rearrange("(t p) k -> p t k", p=128),
                in_=idx_all[:])
            rs_out = [dram.tile([QR, D], F32, tag=f"rs_out{q}",
                                name=f"rs_out{q}") for q in range(NQ)]

            for h in range(NH):
                # ---- load x half (bf16 only; router is pre-computed) ----
                xbf_t = [xbp.tile([128, 4, NT], BF16, tag=f"xbf{k4}",
                                  name=f"xbf{h}_{k4}") for k4 in range(4)]
                for k4 in range(4):
                    nc.sync.dma_start(
                        out=xbf_t[k4][:],
                        in_=xLbf[h * 128:(h + 1) * 128,
                                 k4 * 4 * NT:(k4 + 1) * 4 * NT].rearrange(
                            "p (k j) -> p k j", k=4))

                def xbf(k, cs):
                    return xbf_t[k // 4][:, k % 4, cs]

                # ---- gate/up + swiglu -> hh (bf16; routed comb-scaled) ----
                hh = big.tile([128, IT, NT], BF16, tag="hh")
                hhs = big.tile([128, IST, NT], BF16, tag="hhs")
                for i in range(IT + IST):
                    shared = i >= IT
                    isl = i - IT if shared else i
                    gsrc, usrc = (wsgL, wsuL) if shared else (weL, wuL)
                    wg_t = wstr.tile([128, DT, 128], BF16, tag="wg_t")
                    wu_t = wstr.tile([128, DT, 128], BF16, tag="wu_t")
                    rsl = slice(isl * 128, (isl + 1) * 128)
                    for k4 in range(4):
                        ks = slice(k4 * 4, (k4 + 1) * 4)
                        csl = slice(k4 * 4 * 128, (k4 + 1) * 4 * 128)
                        nc.sync.dma_start(
                            out=wg_t[:, ks, :],
                            in_=gsrc[rsl, csl].rearrange("p (k j) -> p k j", k=4))
                        nc.sync.dma_start(
                            out=wu_t[:, ks, :],
                            in_=usrc[rsl, csl].rearrange("p (k j) -> p k j", k=4))
                    for c in range(NT // 512):
                        cs = slice(c * 512, (c + 1) * 512)
                        pg = psf.tile([128, 512], F32, tag="pg")
                        pu = psf.tile([128, 512], F32, tag="pu")
                        for k in range(DT):
                            nc.tensor.matmul(out=pg[:],
                                             lhsT=wg_t[:, k, :],
                                             rhs=xbf(k, cs),
                                             start=(k == 0), stop=(k == DT - 1))
                        for k in range(DT):
                            nc.tensor.matmul(out=pu[:],
                                             lhsT=wu_t[:, k, :],
                                             rhs=xbf(k, cs),
                                             start=(k == 0), stop=(k == DT - 1))
                        hg = sm.tile([128, 512], F32, tag="hg")
                        nc.scalar.activation(out=hg[:], in_=pg[:],
                                             func=ACT.Silu)
                        dst = hhs if shared else hh
                        nc.vector.tensor_mul(out=dst[:, isl, cs],
                                             in0=hg[:], in1=pu[:])

                # ---- down-proj per 512-token group; RS per quarter ----
                for grp in range(2):
                    q = h * 2 + grp
                    tts = (0, 1, 2, 3)
                    if True:
                      for c in range(D // 512):
                        cs = slice(c * 512, (c + 1) * 512)
                        # shared expert slice -> ysh
                        sps = [psy.tile([128, 512], F32, tag=f"y{t}",
                                        name=f"s_{q}_{c}_{t}")
                               for t in tts]
                        for isl in range(IST):
                            wd_s = wdp.tile([128, 512], BF16, tag="wd_t")
                            nc.gpsimd.dma_start(
                                out=wd_s[:],
                                in_=wsdL[isl * 128:(isl + 1) * 128, cs])
                            for j, tt in enumerate(tts):
                                t = grp * 4 + tt
                                nc.tensor.matmul(
                                    out=sps[j][:],
                                    lhsT=hhs[:, isl, t * 128:(t + 1) * 128],
                                    rhs=wd_s[:],
                                    start=(isl == 0), stop=(isl == IST - 1))
                        yshs = []
                        for j, tt in enumerate(tts):
                            ysh = ysp.tile([128, 512], F32, tag=f"ysh{tt}",
                                           name=f"ysh_{q}_{c}_{tt}")
                            nc.vector.tensor_copy(out=ysh[:], in_=sps[j][:])
                            yshs.append(ysh)
                        # routed expert, comb-scaled at drain, += shared
                        yps = [psy.tile([128, 512], F32, tag=f"y{t}",
                                        name=f"y_{q}_{c}_{t}")
                               for t in tts]
                        for isl in range(IT):
                            wd_t = wdp.tile([128, 512], BF16, tag="wd_t")
                            nc.gpsimd.dma_start(
                                out=wd_t[:],
                                in_=wdL[isl * 128:(isl + 1) * 128, cs])
                            for j, tt in enumerate(tts):
                                t = grp * 4 + tt
                                nc.tensor.matmul(
                                    out=yps[j][:],
                                    lhsT=hh[:, isl, t * 128:(t + 1) * 128],
                                    rhs=wd_t[:],
                                    start=(isl == 0), stop=(isl == IT - 1))
                        for j, tt in enumerate(tts):
                            t = grp * 4 + tt
                            ysb = ysp.tile([128, 512], F32, tag="ysb")
                            ta = h * TT + t
                            nc.vector.scalar_tensor_tensor(
                                out=ysb[:], in0=yps[j][:],
                                scalar=comb_all[:, ta:ta + 1], in1=yshs[j][:],
                                op0=mybir.AluOpType.mult,
                                op1=mybir.AluOpType.add)
                            nc.scalar.dma_start(
                                out=rs_in[q][tt * 128:(tt + 1) * 128, cs],
                                in_=ysb[:])
                    if True:
                        nc.gpsimd.collective_compute(
                            "ReduceScatter",
                            ALU.add,
                            replica_groups=[list(range(NCORES))],
                            ins=[rs_in[q][:].opt()],
                            outs=[rs_out[q][:].opt()],
                        )

            for q in range(NQ):
                nc.sync.dma_start(
                    out=out_shard[q * QR:(q + 1) * QR, :],
                    in_=rs_out[q][:])

    nc.compile()
    _CACHE["nc"] = nc
    return nc


def _tile_gateup(wT_bf):
    # [D, Icols] bf16 -> [Icols(it-major), DT*128] st. row it*128+j, col k*128+p
    # equals wT[k*128+p, it*128+j]
    d, icols = wT_bf.shape
    it = icols // 128
    a = wT_bf.reshape(DT, 128, it, 128).transpose(2, 1, 0, 3)  # [it, p, k, j]
    return np.ascontiguousarray(a.reshape(it * 128, DT * 128))


def _make_in_maps(x, gate_w, expert_bias, shared_gate, shared_up, shared_down,
                  exp_gate, exp_up, exp_down):
    x2 = np.ascontiguousarray(np.asarray(x, np.float32).reshape(N, D))
    xT_v = x2.T  # [D, N]
    # xL[h*128+p, k*NT+j] = xT[k*128+p, h*NT+j]
    xL_v = np.ascontiguousarray(
        xT_v.reshape(DT, 128, NH, NT).transpose(2, 1, 0, 3).reshape(
            NH * 128, DT * NT))
    xLbf_v = xL_v.astype(BF)
    wgT_v = np.ascontiguousarray(np.asarray(gate_w, np.float32).T)
    ebias_v = np.ascontiguousarray(np.asarray(expert_bias, np.float32)
                                   .reshape(1, E))
    iota_v = np.arange(E, dtype=np.float32).reshape(1, E)

    sg = np.asarray(shared_gate, np.float32)
    su = np.asarray(shared_up, np.float32)
    sd = np.asarray(shared_down, np.float32)

    in_maps = []
    for e in range(NCORES):
        sl = slice(e * IS, (e + 1) * IS)
        esel_v = np.zeros((1, E), np.float32)
        esel_v[0, e] = 1.0

        weT = np.asarray(exp_gate[e], np.float32).T.astype(BF)   # [D, I]
        wuT = np.asarray(exp_up[e], np.float32).T.astype(BF)
        wdT = np.asarray(exp_down[e], np.float32).T.astype(BF)   # [I, D]

        wsgT = np.zeros((D, ISP), BF)
        wsgT[:, :IS] = sg[sl, :].T.astype(BF)
        wsuT = np.zeros((D, ISP), BF)
        wsuT[:, :IS] = su[sl, :].T.astype(BF)
        wsd_p = np.zeros((ISP, D), BF)
        wsd_p[:IS, :] = sd[:, sl].T.astype(BF)

        xro_v = np.ascontiguousarray(
            xT_v[:, e * 256:(e + 1) * 256].reshape(DT, 128, 256)
            .transpose(1, 0, 2).reshape(128, DT * 256))
        in_maps.append({
            "xro": xro_v,
            "ecol": np.full((1, 1), float(e), np.float32),
            "xLbf": xLbf_v,
            "wgT": wgT_v,
            "ebias": ebias_v,
            "esel": esel_v,
            "iota8": iota_v,
            "weL": _tile_gateup(weT),
            "wuL": _tile_gateup(wuT),
            "wdL": np.ascontiguousarray(wdT),
            "wsgL": _tile_gateup(wsgT),
            "wsuL": _tile_gateup(wsuT),
            "wsdL": np.ascontiguousarray(wsd_p),
        })
    return in_maps


def _assemble(results):
    out = np.empty((N, D), np.float32)
    for q in range(NQ):
        for r in range(NCORES):
            out[q * (N // NQ) + r * QR:q * (N // NQ) + (r + 1) * QR, :] = \
                results[r]["out_shard"][q * QR:(q + 1) * QR, :]
    idx = results[0]["out_idx"].astype(np.int32)
    return out.reshape(B, S, D), idx.reshape(B, S, TOPK)


def kernel(x, gate_w, expert_bias, shared_gate, shared_up, shared_down,
           exp_gate, exp_up, exp_down, _trace=False):
    nc = _build()
    in_maps = _make_in_maps(x, gate_w, expert_bias, shared_gate, shared_up,
                            shared_down, exp_gate, exp_up, exp_down)
    last_err = None
    for attempt in range(3):
        try:
            res = run_bass_kernel_spmd(nc, in_maps,
                                       core_ids=list(range(NCORES)),
                                       trace=_trace)
            break
        except Exception as e:  # transient NRT_EXEC_UNIT_UNRECOVERABLE
            last_err = e
            if "UNRECOVERABLE" not in str(e) and "UNAVAILABLE" not in str(e):
                raise
            import time
            time.sleep(10)
    else:
        raise last_err
    _CACHE["last_result"] = res
    return _assemble(res.results)


# revision 32
# speedup vs baseline: 1.0362x; 1.0024x over previous
"""DeepSeekMoE layer on 8 Trainium2 NeuronCores (expert-parallel).

Hardcoded problem shapes: B=2, S=1024, D=2048, I=1408, E=8, TOPK=2.

Sharding: core e holds routed expert e (dense over all 2048 tokens)
plus a 176-wide I-slice of the shared expert (zero-padded to 256).
Routed output is pre-scaled by the per-token combine weight (folded
into the SwiGLU product), so routed and shared partials accumulate in
the same PSUM; ReduceScatters over the 8 cores (one per 512-token
quarter, overlapped with compute) sum the partials and hand core j its
token rows. The router runs replicated on every core in true fp32 so
the top-2 selection matches the fp32 reference bit-for-bit; the FFN
runs in bf16 with fp32 PSUM accumulation.

Host side only does value-independent layout prep (weight transposes /
tiling / bf16 casts, x transpose) and shard concatenation.
"""

import numpy as np
import ml_dtypes

import concourse.bacc as bacc
import concourse.mybir as mybir
import concourse.tile as tile
from concourse.bass_utils import run_bass_kernel_spmd
from concourse.masks import make_identity

F32 = mybir.dt.float32
BF16 = mybir.dt.bfloat16
I32 = mybir.dt.int32
U32 = mybir.dt.uint32
BF = ml_dtypes.bfloat16

B, S, D, I, E, TOPK = 2, 1024, 2048, 1408, 8, 2
N = B * S                    # 2048 tokens
NCORES = 8
IS = I // NCORES             # shared I slice per core (176)
ISP = 256                    # padded shared slice
NH = 2                       # token halves
NT = N // NH                 # tokens per half (1024)
TT = NT // 128               # token tiles per half (8)
DT = D // 128                # 16
IT = I // 128                # 11 (routed i tiles)
IST = ISP // 128             # 2  (shared i tiles, padded)
NQ = 4                       # reduce-scatter quarters (512 tokens each)
QR = N // NQ // NCORES       # rows per core per quarter (64)

_CACHE = {}


def _build():
    if "nc" in _CACHE:
        return _CACHE["nc"]

    nc = bacc.Bacc("TRN2", target_bir_lowering=False, debug=False,
                   num_devices=NCORES)

    xLbf = nc.dram_tensor("xLbf", [NH * 128, DT * NT], BF16, kind="ExternalInput")
    xro = nc.dram_tensor("xro", [128, DT * 256], F32, kind="ExternalInput")
    ecol = nc.dram_tensor("ecol", [1, 1], F32, kind="ExternalInput")
    wgT = nc.dram_tensor("wgT", [D, E], F32, kind="ExternalInput")
    ebias = nc.dram_tensor("ebias", [1, E], F32, kind="ExternalInput")
    esel = nc.dram_tensor("esel", [1, E], F32, kind="ExternalInput")
    iota8 = nc.dram_tensor("iota8", [1, E], F32, kind="ExternalInput")
    # routed expert weights, i-tile-major bf16 layouts
    weL = nc.dram_tensor("weL", [IT * 128, DT * 128], BF16, kind="ExternalInput")
    wuL = nc.dram_tensor("wuL", [IT * 128, DT * 128], BF16, kind="ExternalInput")
    wdL = nc.dram_tensor("wdL", [IT * 128, D], BF16, kind="ExternalInput")
    # shared expert slice (padded to ISP rows)
    wsgL = nc.dram_tensor("wsgL", [IST * 128, DT * 128], BF16, kind="ExternalInput")
    wsuL = nc.dram_tensor("wsuL", [IST * 128, DT * 128], BF16, kind="ExternalInput")
    wsdL = nc.dram_tensor("wsdL", [IST * 128, D], BF16, kind="ExternalInput")

    out_shard = nc.dram_tensor("out_shard", [N // NCORES, D], F32,
                               kind="ExternalOutput")
    out_idx = nc.dram_tensor("out_idx", [N, TOPK], I32, kind="ExternalOutput")

    ACT = mybir.ActivationFunctionType
    ALU = mybir.AluOpType

    with tile.TileContext(nc) as tc:
        with (
            tc.tile_pool(name="big", bufs=1) as big,
            tc.tile_pool(name="xbp", bufs=2) as xbp,
            tc.tile_pool(name="wstr", bufs=3) as wstr,
            tc.tile_pool(name="wdp", bufs=8) as wdp,
            tc.tile_pool(name="ysp", bufs=4) as ysp,
            tc.tile_pool(name="sm", bufs=2) as sm,
            tc.tile_pool(name="psf", bufs=2, space="PSUM") as psf,
            tc.tile_pool(name="psy", bufs=1, space="PSUM") as psy,
            tc.tile_pool(name="dram", bufs=1, space="DRAM") as dram,
        ):
            ident = big.tile([128, 128], F32, tag="ident")
            make_identity(nc, ident[:])
            bias_b = big.tile([128, E], F32, tag="bias_b")
            nc.sync.dma_start(out=bias_b[:], in_=ebias[:, :].to_broadcast((128, E)))
            esel_b = big.tile([128, E], F32, tag="esel_b")
            nc.sync.dma_start(out=esel_b[:], in_=esel[:, :].to_broadcast((128, E)))
            iota_b = big.tile([128, E], F32, tag="iota_b")
            nc.sync.dma_start(out=iota_b[:], in_=iota8[:, :].to_broadcast((128, E)))
            wg_sb = big.tile([128, DT, E], F32, tag="wg_sb")
            nc.sync.dma_start(
                out=wg_sb[:],
                in_=wgT[:, :].rearrange("(k p) e -> p k e", p=128))

            rs_in = [dram.tile([N // NQ, D], F32, tag=f"rs_in{q}",
                               name=f"rs_in{q}") for q in range(NQ)]
            ag_in = dram.tile([256, 4], F32, tag="ag_in", name="ag_in")
            ag_out = dram.tile([N, 4], F32, tag="ag_out", name="ag_out")

            # ---- sharded router: fp32 scores for own 256 tokens ----
            ec_b = big.tile([128, 1], F32, tag="ec_b")
            nc.sync.dma_start(out=ec_b[:], in_=ecol[:, :].to_broadcast((128, 1)))
            xro_sb = big.tile([128, DT, 256], F32, tag="xro_sb")
            nc.sync.dma_start(
                out=xro_sb[:],
                in_=xro[:, :].rearrange("p (k j) -> p k j", k=DT))
            zps = psf.tile([E, 256], F32, tag="pg")
            for k in range(DT):
                nc.tensor.matmul(out=zps[:], lhsT=wg_sb[:, k, :],
                                 rhs=xro_sb[:, k, :],
                                 start=(k == 0), stop=(k == DT - 1))
            zsb = sm.tile([E, 256], F32, tag="zsb")
            nc.scalar.activation(out=zsb[:], in_=zps[:], func=ACT.Sigmoid)
            pack = sm.tile([128, 2, 4], F32, tag="pack")
            for t in range(2):
                tp = psf.tile([128, E], F32, tag="pu")
                nc.tensor.transpose(out=tp[:],
                                    in_=zsb[:, t * 128:(t + 1) * 128],
                                    identity=ident[:E, :E])
                sc = sm.tile([128, E], F32, tag="sc")
                nc.vector.tensor_copy(out=sc[:], in_=tp[:])
                sel = sm.tile([128, E], F32, tag="sel")
                nc.vector.tensor_add(out=sel[:], in0=sc[:], in1=bias_b[:])
                mx = sm.tile([128, E], F32, tag="mx")
                mi = sm.tile([128, E], U32, tag="mi")
                nc.vector.max(out=mx[:], in_=sel[:])
                nc.vector.max_index(out=mi[:], in_max=mx[:], in_values=sel[:])
                mif = sm.tile([128, E], F32, tag="mif")
                nc.vector.tensor_copy(out=mif[:], in_=mi[:])
                nc.vector.tensor_copy(out=pack[:, t, 0:2], in_=mif[:, 0:2])
                oh1 = sm.tile([128, E], F32, tag="oh1")
                oh2 = sm.tile([128, E], F32, tag="oh2")
                nc.vector.tensor_tensor(
                    out=oh1[:], in0=mif[:, 0:1].to_broadcast((128, E)),
                    in1=iota_b[:], op=ALU.is_equal)
                nc.vector.tensor_tensor(
                    out=oh2[:], in0=mif[:, 1:2].to_broadcast((128, E)),
                    in1=iota_b[:], op=ALU.is_equal)
                prod = sm.tile([128, E], F32, tag="prod")
                w1 = sm.tile([128, 1], F32, tag="w1")
                w2 = sm.tile([128, 1], F32, tag="w2")
                nc.vector.tensor_mul(out=prod[:], in0=oh1[:], in1=sc[:])
                nc.vector.tensor_reduce(out=w1[:], in_=prod[:],
                                        axis=mybir.AxisListType.X, op=ALU.add)
                nc.vector.tensor_mul(out=prod[:], in0=oh2[:], in1=sc[:])
                nc.vector.tensor_reduce(out=w2[:], in_=prod[:],
                                        axis=mybir.AxisListType.X, op=ALU.add)
                ssum = sm.tile([128, 1], F32, tag="ssum")
                nc.vector.tensor_add(out=ssum[:], in0=w1[:], in1=w2[:])
                sinv = sm.tile([128, 1], F32, tag="sinv")
                nc.vector.reciprocal(out=sinv[:], in_=ssum[:])
                nc.vector.tensor_mul(out=pack[:, t, 2:3], in0=w1[:], in1=sinv[:])
                nc.vector.tensor_mul(out=pack[:, t, 3:4], in0=w2[:], in1=sinv[:])
            nc.scalar.dma_start(
                out=ag_in[:].rearrange("(t p) c -> p t c", p=128),
                in_=pack[:])
            nc.gpsimd.collective_compute(
                "AllGather", ALU.bypass,
                replica_groups=[list(range(NCORES))],
                ins=[ag_in[:].opt()],
                outs=[ag_out[:].opt()])
            pk = sm.tile([128, N // 128, 4], F32, tag="pk")
            nc.sync.dma_start(
                out=pk[:],
                in_=ag_out[:].rearrange("(t p) c -> p t c", p=128))
            comb_all = big.tile([128, N // 128], F32, tag="comb_all")
            idx_all = sm.tile([128, N // 128, TOPK], I32, tag="idx_all")
            for t in range(N // 128):
                eq1 = sm.tile([128, 1], F32, tag="eq1")
                eq2 = sm.tile([128, 1], F32, tag="eq2")
                nc.vector.tensor_scalar(out=eq1[:], in0=pk[:, t, 0:1],
                                        scalar1=ec_b[:], scalar2=None,
                                        op0=ALU.is_equal)
                nc.vector.tensor_scalar(out=eq2[:], in0=pk[:, t, 1:2],
                                        scalar1=ec_b[:], scalar2=None,
                                        op0=ALU.is_equal)
                m1 = sm.tile([128, 1], F32, tag="m1")
                m2 = sm.tile([128, 1], F32, tag="m2")
                nc.vector.tensor_mul(out=m1[:], in0=eq1[:], in1=pk[:, t, 2:3])
                nc.vector.tensor_mul(out=m2[:], in0=eq2[:], in1=pk[:, t, 3:4])
                nc.vector.tensor_add(out=comb_all[:, t:t + 1], in0=m1[:],
                                     in1=m2[:])
                nc.vector.tensor_copy(out=idx_all[:, t, :], in_=pk[:, t, 0:2])
            nc.sync.dma_start(
                out=out_idx[:, :].rearrange("(t p) k -> p t k", p=128),
                in_=idx_all[:])
            rs_out = [dram.tile([QR, D], F32, tag=f"rs_out{q}",
                                name=f"rs_out{q}") for q in range(NQ)]

            for h in range(NH):
                # ---- load x half (bf16 only; router is pre-computed) ----
                xbf_t = [xbp.tile([128, 4, NT], BF16, tag=f"xbf{k4}",
                                  name=f"xbf{h}_{k4}") for k4 in range(4)]
                for k4 in range(4):
                    nc.sync.dma_start(
                        out=xbf_t[k4][:],
                        in_=xLbf[h * 128:(h + 1) * 128,
                                 k4 * 4 * NT:(k4 + 1) * 4 * NT].rearrange(
                            "p (k j) -> p k j", k=4))

                def xbf(k, cs):
                    return xbf_t[k // 4][:, k % 4, cs]

                # ---- gate/up + swiglu -> hh (bf16; routed comb-scaled) ----
                hh = big.tile([128, IT, NT], BF16, tag="hh")
                hhs = big.tile([128, IST, NT], BF16, tag="hhs")
                for i in range(IT + IST):
                    shared = i >= IT
                    isl = i - IT if shared else i
                    gsrc, usrc = (wsgL, wsuL) if shared else (weL, wuL)
                    wg_t = wstr.tile([128, DT, 128], BF16, tag="wg_t")
                    wu_t = wstr.tile([128, DT, 128], BF16, tag="wu_t")
                    rsl = slice(isl * 128, (isl + 1) * 128)
                    for k4 in range(4):
                        ks = slice(k4 * 4, (k4 + 1) * 4)
                        csl = slice(k4 * 4 * 128, (k4 + 1) * 4 * 128)
                        nc.sync.dma_start(
                            out=wg_t[:, ks, :],
                            in_=gsrc[rsl, csl].rearrange("p (k j) -> p k j", k=4))
                        nc.sync.dma_start(
                            out=wu_t[:, ks, :],
                            in_=usrc[rsl, csl].rearrange("p (k j) -> p k j", k=4))
                    for c in range(NT // 512):
                        cs = slice(c * 512, (c + 1) * 512)
                        pg = psf.tile([128, 512], F32, tag="pg")
                        pu = psf.tile([128, 512], F32, tag="pu")
                        for k in range(DT):
                            nc.tensor.matmul(out=pg[:],
                                             lhsT=wg_t[:, k, :],
                                             rhs=xbf(k, cs),
                                             start=(k == 0), stop=(k == DT - 1))
                        for k in range(DT):
                            nc.tensor.matmul(out=pu[:],
                                             lhsT=wu_t[:, k, :],
                                             rhs=xbf(k, cs),
                                             start=(k == 0), stop=(k == DT - 1))
                        hg = sm.tile([128, 512], F32, tag="hg")
                        nc.scalar.activation(out=hg[:], in_=pg[:],
                                             func=ACT.Silu)
                        dst = hhs if shared else hh
                        nc.vector.tensor_mul(out=dst[:, isl, cs],
                                             in0=hg[:], in1=pu[:])

                # ---- down-proj per 512-token group; RS per quarter ----
                for grp in range(2):
                    q = h * 2 + grp
                    tts = (0, 1, 2, 3)
                    if True:
                      for c in range(D // 512):
                        cs = slice(c * 512, (c + 1) * 512)
                        # shared expert slice -> ysh
                        sps = [psy.tile([128, 512], F32, tag=f"y{t}",
                                        name=f"s_{q}_{c}_{t}")
                               for t in tts]
                        for isl in range(IST):
                            wd_s = wdp.tile([128, 512], BF16, tag="wd_t")
                            (nc.gpsimd if isl % 2 == 0 else nc.scalar).dma_start(
                                out=wd_s[:],
                                in_=wsdL[isl * 128:(isl + 1) * 128, cs])
                            for j, tt in enumerate(tts):
                                t = grp * 4 + tt
                                nc.tensor.matmul(
                                    out=sps[j][:],
                                    lhsT=hhs[:, isl, t * 128:(t + 1) * 128],
                                    rhs=wd_s[:],
                                    start=(isl == 0), stop=(isl == IST - 1))
                        yshs = []
                        for j, tt in enumerate(tts):
                            ysh = ysp.tile([128, 512], F32, tag=f"ysh{tt}",
                                           name=f"ysh_{q}_{c}_{tt}")
                            nc.vector.tensor_copy(out=ysh[:], in_=sps[j][:])
                            yshs.append(ysh)
                        # routed expert, comb-scaled at drain, += shared
                        yps = [psy.tile([128, 512], F32, tag=f"y{t}",
                                        name=f"y_{q}_{c}_{t}")
                               for t in tts]
                        for isl in range(IT):
                            wd_t = wdp.tile([128, 512], BF16, tag="wd_t")
                            (nc.gpsimd if isl % 2 == 0 else nc.scalar).dma_start(
                                out=wd_t[:],
                                in_=wdL[isl * 128:(isl + 1) * 128, cs])
                            for j, tt in enumerate(tts):
                                t = grp * 4 + tt
                                nc.tensor.matmul(
                                    out=yps[j][:],
                                    lhsT=hh[:, isl, t * 128:(t + 1) * 128],
                                    rhs=wd_t[:],
                                    start=(isl == 0), stop=(isl == IT - 1))
                        for j, tt in enumerate(tts):
                            t = grp * 4 + tt
                            ysb = ysp.tile([128, 512], F32, tag="ysb")
                            ta = h * TT + t
                            nc.vector.scalar_tensor_tensor(
                                out=ysb[:], in0=yps[j][:],
                                scalar=comb_all[:, ta:ta + 1], in1=yshs[j][:],
                                op0=mybir.AluOpType.mult,
                                op1=mybir.AluOpType.add)
                            nc.scalar.dma_start(
                                out=rs_in[q][tt * 128:(tt + 1) * 128, cs],
                                in_=ysb[:])
                    if True:
                        nc.gpsimd.collective_compute(
                            "ReduceScatter",
                            ALU.add,
                            replica_groups=[list(range(NCORES))],
                            ins=[rs_in[q][:].opt()],
                            outs=[rs_out[q][:].opt()],
                        )

            for q in range(NQ):
                nc.sync.dma_start(
                    out=out_shard[q * QR:(q + 1) * QR, :],
                    in_=rs_out[q][:])

    nc.compile()
    _CACHE["nc"] = nc
    return nc


def _tile_gateup(wT_bf):
    # [D, Icols] bf16 -> [Icols(it-major), DT*128] st. row it*128+j, col k*128+p
    # equals wT[k*128+p, it*128+j]
    d, icols = wT_bf.shape
    it = icols // 128
    a = wT_bf.reshape(DT, 128, it, 128).transpose(2, 1, 0, 3)  # [it, p, k, j]
    return np.ascontiguousarray(a.reshape(it * 128, DT * 128))


def _make_in_maps(x, gate_w, expert_bias, shared_gate, shared_up, shared_down,
                  exp_gate, exp_up, exp_down):
    x2 = np.ascontiguousarray(np.asarray(x, np.float32).reshape(N, D))
    xT_v = x2.T  # [D, N]
    # xL[h*128+p, k*NT+j] = xT[k*128+p, h*NT+j]
    xL_v = np.ascontiguousarray(
        xT_v.reshape(DT, 128, NH, NT).transpose(2, 1, 0, 3).reshape(
            NH * 128, DT * NT))
    xLbf_v = xL_v.astype(BF)
    wgT_v = np.ascontiguousarray(np.asarray(gate_w, np.float32).T)
    ebias_v = np.ascontiguousarray(np.asarray(expert_bias, np.float32)
                                   .reshape(1, E))
    iota_v = np.arange(E, dtype=np.float32).reshape(1, E)

    sg = np.asarray(shared_gate, np.float32)
    su = np.asarray(shared_up, np.float32)
    sd = np.asarray(shared_down, np.float32)

    in_maps = []
    for e in range(NCORES):
        sl = slice(e * IS, (e + 1) * IS)
        esel_v = np.zeros((1, E), np.float32)
        esel_v[0, e] = 1.0

        weT = np.asarray(exp_gate[e], np.float32).T.astype(BF)   # [D, I]
        wuT = np.asarray(exp_up[e], np.float32).T.astype(BF)
        wdT = np.asarray(exp_down[e], np.float32).T.astype(BF)   # [I, D]

        wsgT = np.zeros((D, ISP), BF)
        wsgT[:, :IS] = sg[sl, :].T.astype(BF)
        wsuT = np.zeros((D, ISP), BF)
        wsuT[:, :IS] = su[sl, :].T.astype(BF)
        wsd_p = np.zeros((ISP, D), BF)
        wsd_p[:IS, :] = sd[:, sl].T.astype(BF)

        xro_v = np.ascontiguousarray(
            xT_v[:, e * 256:(e + 1) * 256].reshape(DT, 128, 256)
            .transpose(1, 0, 2).reshape(128, DT * 256))
        in_maps.append({
            "xro": xro_v,
            "ecol": np.full((1, 1), float(e), np.float32),
            "xLbf": xLbf_v,
            "wgT": wgT_v,
            "ebias": ebias_v,
            "esel": esel_v,
            "iota8": iota_v,
            "weL": _tile_gateup(weT),
            "wuL": _tile_gateup(wuT),
            "wdL": np.ascontiguousarray(wdT),
            "wsgL": _tile_gateup(wsgT),
            "wsuL": _tile_gateup(wsuT),
            "wsdL": np.ascontiguousarray(wsd_p),
        })
    return in_maps


def _assemble(results):
    out = np.empty((N, D), np.float32)
    for q in range(NQ):
        for r in range(NCORES):
            out[q * (N // NQ) + r * QR:q * (N // NQ) + (r + 1) * QR, :] = \
                results[r]["out_shard"][q * QR:(q + 1) * QR, :]
    idx = results[0]["out_idx"].astype(np.int32)
    return out.reshape(B, S, D), idx.reshape(B, S, TOPK)


def kernel(x, gate_w, expert_bias, shared_gate, shared_up, shared_down,
           exp_gate, exp_up, exp_down, _trace=False):
    nc = _build()
    in_maps = _make_in_maps(x, gate_w, expert_bias, shared_gate, shared_up,
                            shared_down, exp_gate, exp_up, exp_down)
    last_err = None
    for attempt in range(3):
        try:
            res = run_bass_kernel_spmd(nc, in_maps,
                                       core_ids=list(range(NCORES)),
                                       trace=_trace)
            break
        except Exception as e:  # transient NRT_EXEC_UNIT_UNRECOVERABLE
            last_err = e
            if "UNRECOVERABLE" not in str(e) and "UNAVAILABLE" not in str(e):
                raise
            import time
            time.sleep(10)
    else:
        raise last_err
    _CACHE["last_result"] = res
    return _assemble(res.results)
